# revision 1
# baseline (speedup 1.0000x reference)
"""Causal multi-head attention (B=4, H=16, S=2048, D=64) on 8 Trainium2 cores.

Sharding: B*H = 64 independent attention problems, 8 heads per core.

Per-core design (v2 — split-bf16 matmuls, measured 6x faster than fp32/fp32r):
- Heads in pairs (A at partitions 0:64, B at 64:128); QK matmuls of the two
  heads run concurrently on disjoint PE row groups.
- Q,K,V split into bf16 hi+lo: QK^T = Qhi.Khi + Qhi.Klo + Qlo.Khi (error
  ~1e-5); PV = P.Vhi + P.Vlo with P in bf16 (P rounding error averages out
  in the softmax-weighted sum; denominator uses the same rounded P via the
  ones column of Vhi).
- S^T computed directly in [k, q] layout (lhsT = K^T chunk, rhs = Q^T block)
  so no P transposes are needed; Q^T/K^T produced by DMA xbar transposes
  (bf16) — zero PE cost.
- exp: one ACT instruction per 4-bank PSUM group [128, 2048], scale=1/8
  folded in; causal masking via multiplicative 0/1 bf16 masks on diagonal
  chunks only.
- PV accumulates in PSUM [65, 512]; the ones column of Vhi yields softmax
  denominators for free. Finalize: PE transpose [65,512] -> 4x [128,65],
  reciprocal + per-partition scalar multiply, contiguous DMA out.
"""
import numpy as np

B, H, S, D = 4, 16, 2048, 64
NCORES = 8
HPC = B * H // NCORES      # 8 heads per core
P = 128
QBLK = 512
NT = S // P                # 16 k-chunks / q-tiles per head
NBLK = S // QBLK           # 4 q blocks
NPAIR = HPC // 2           # 4 head pairs per core

_cache = {}


def _build(reps=1):
    from contextlib import ExitStack
    import concourse.bacc as bacc
    import concourse.tile as tile
    import concourse.mybir as mybir
    from concourse.masks import make_identity

    f32 = mybir.dt.float32
    bf16 = mybir.dt.float16  # fp16: same PE rate as bf16, 4x less rounding
    AF = mybir.ActivationFunctionType

    nc = bacc.Bacc("TRN2", target_bir_lowering=False, debug=False,
                   num_devices=NCORES)
    Qd = nc.dram_tensor("Q", (HPC, S, D), f32, kind="ExternalInput")
    Kd = nc.dram_tensor("K", (HPC, S, D), f32, kind="ExternalInput")
    Vd = nc.dram_tensor("V", (HPC, S, D), f32, kind="ExternalInput")
    Od = nc.dram_tensor("O", (HPC, S, D), f32, kind="ExternalOutput")

    with tile.TileContext(nc) as tc, ExitStack() as ctx:
        consts = ctx.enter_context(tc.tile_pool(name="consts", bufs=1))
        raw = ctx.enter_context(tc.tile_pool(name="raw", bufs=2))
        qk = ctx.enter_context(tc.tile_pool(name="qk", bufs=2))
        ptp = ctx.enter_context(tc.tile_pool(name="ptp", bufs=2))
        fin = ctx.enter_context(tc.tile_pool(name="fin", bufs=2))
        st_ps = ctx.enter_context(tc.tile_pool(name="st_ps", bufs=1, space="PSUM"))
        acc_ps = ctx.enter_context(tc.tile_pool(name="acc_ps", bufs=2, space="PSUM"))
        tr_ps = ctx.enter_context(tc.tile_pool(name="tr_ps", bufs=2, space="PSUM"))

        ident = consts.tile([128, 128], f32)
        make_identity(nc, ident[:])
        onesf = consts.tile([128, 2 * NT], f32)
        nc.vector.memset(onesf[:], 1.0)
        # masks[j][r, c] = 1.0 iff c - r - j*128 >= 0 (valid); only cols
        # 0:(j+1)*128 are ever used (cols beyond are fully valid).
        masks = []
        for j in range(4):
            mjf = consts.tile([128, 512], f32, name=f"maskf{j}")
            nc.gpsimd.memset(mjf[:], 1.0)
            nc.gpsimd.affine_select(
                out=mjf[:], in_=mjf[:], compare_op=mybir.AluOpType.is_ge,
                fill=0.0, base=-128 * j, pattern=[[1, 512]],
                channel_multiplier=-1)
            mj = consts.tile([128, 512], bf16, name=f"mask{j}")
            nc.vector.tensor_copy(mj[:], mjf[:])
            masks.append(mj)

        for rep in range(reps):
          for pair in range(NPAIR):
            hA = 2 * pair
            # ---- loads ----
            qraw = raw.tile([128, NT * 2 * 64], f32)
            kraw = raw.tile([128, NT * 2 * 64], f32)
            vf = raw.tile([128, 2 * NT * 64], f32)
            for hh in range(2):
                nc.sync.dma_start(
                    qraw[:].rearrange("p (n h d) -> p n h d", n=NT, h=2)[:, :, hh, :],
                    Qd[hA + hh, :, :].rearrange("(n p) d -> p n d", p=P))
                nc.sync.dma_start(
                    kraw[:].rearrange("p (n h d) -> p n h d", n=NT, h=2)[:, :, hh, :],
                    Kd[hA + hh, :, :].rearrange("(n p) d -> p n d", p=P))
                nc.sync.dma_start(
                    vf[:].rearrange("p (h n d) -> p h n d", h=2, n=NT)[:, hh, :, :],
                    Vd[hA + hh, :, :].rearrange("(n p) d -> p n d", p=P))

            # ---- bf16 hi/lo splits ----
            qhi = raw.tile([128, NT * 128], bf16)
            qlo = raw.tile([128, NT * 128], bf16)
            khi = raw.tile([128, NT * 128], bf16)
            klo = raw.tile([128, NT * 128], bf16)
            for src, hi, lo in ((qraw, qhi, qlo), (kraw, khi, klo)):
                nc.vector.tensor_copy(hi[:], src[:])
                nc.vector.tensor_sub(lo[:], src[:], hi[:])
            vhi = raw.tile([128, 2 * NT * 65], bf16)
            vlo = raw.tile([128, 2 * NT * 65], bf16)
            vhi_v = vhi[:].rearrange("p (h n e) -> p h n e", h=2, n=NT)
            vlo_v = vlo[:].rearrange("p (h n e) -> p h n e", h=2, n=NT)
            vf_v = vf[:].rearrange("p (h n d) -> p h n d", h=2, n=NT)
            nc.vector.tensor_copy(vhi_v[:, :, :, 0:64], vf_v)
            nc.vector.tensor_sub(vlo_v[:, :, :, 0:64], vf_v, vhi_v[:, :, :, 0:64])
            nc.vector.tensor_copy(
                vhi_v[:, :, :, 64:65],
                onesf[:].rearrange("p (h n) -> p h n", h=2)[:, :, :, None])
            nc.vector.memset(vlo_v[:, :, :, 64:65], 0.0)

            # ---- DMA xbar transposes: stacked [128,128] per (half, tile) ----
            # row 0:64 = head A (d), 64:128 = head B after transpose
            qthi = qk.tile([128, S], bf16)
            qtlo = qk.tile([128, S], bf16)
            kthi = qk.tile([128, S], bf16)
            ktlo = qk.tile([128, S], bf16)
            for t in range(NT):
                for src, dst in ((qhi, qthi), (qlo, qtlo), (khi, kthi),
                                 (klo, ktlo)):
                    nc.sync.dma_start_transpose(
                        dst[:, t * 128:(t + 1) * 128],
                        src[:].rearrange("p (n c) -> p n c", n=NT)[:, t, :])

            # ---- attention blocks ----
            for b in range(NBLK):
                nchunks = 4 * b + 4
                accs = [acc_ps.tile([65, 512], f32, tag="acc", name=f"acc{hh}")
                        for hh in range(2)]
                for g in range(nchunks // 2):
                    cpair = (2 * g, 2 * g + 1)
                    st = st_ps.tile([128, 2048], f32, tag="st", name="st")
                    quads = [(cc, hh) for cc in cpair for hh in range(2)]

                    def slot(cc, hh):
                        return (cc - cpair[0]) * 2 + hh

                    for cc in cpair:
                        terms = [(kthi, qthi), (kthi, qtlo), (ktlo, qthi)]
                        for ti, (ktt, qtt) in enumerate(terms):
                            for hh in range(2):
                                i = slot(cc, hh)
                                nc.tensor.matmul(
                                    st[:, i * 512:(i + 1) * 512],
                                    ktt[hh * 64:(hh + 1) * 64,
                                        cc * 128:(cc + 1) * 128],
                                    qtt[hh * 64:(hh + 1) * 64,
                                        b * 512:(b + 1) * 512],
                                    start=(ti == 0), stop=(ti == len(terms) - 1))
                    pt = ptp.tile([128, 2048], bf16, tag="pt", name="pt")
                    nc.scalar.activation(pt[:], st[:], AF.Exp, scale=0.125)
                    for cc, hh in quads:
                        i = slot(cc, hh)
                        j = cc - 4 * b
                        if j >= 0:  # diagonal chunk: zero invalid region
                            w = (j + 1) * 128
                            nc.vector.tensor_mul(
                                pt[:, i * 512:i * 512 + w],
                                pt[:, i * 512:i * 512 + w],
                                masks[j][:, 0:w])
                    for cc, hh in quads:
                        i = slot(cc, hh)
                        for vt, first, last in ((vhi_v, cc == 0, False),
                                                (vlo_v, False,
                                                 cc == nchunks - 1)):
                            nc.tensor.matmul(
                                accs[hh][:],
                                vt[:, hh, cc, :],
                                pt[:, i * 512:(i + 1) * 512],
                                start=first, stop=last)

                # ---- finalize block: transpose + normalize + store ----
                for hh in range(2):
                    osb = fin.tile([65, 512], f32, tag="osb", name="osb")
                    nc.vector.tensor_copy(osb[:], accs[hh][:])
                    ot = tr_ps.tile([128, 260], f32, tag="tr", name="ot")
                    for j in range(4):
                        nc.tensor.transpose(
                            ot[:, j * 65:(j + 1) * 65],
                            osb[:, j * 128:(j + 1) * 128],
                            ident[0:65, 0:65])
                    recip = fin.tile([128, 4], f32, tag="recip", name="recip")
                    nc.vector.reciprocal(
                        recip[:],
                        ot[:].rearrange("p (j e) -> p j e", e=65)[:, :, 64])
                    o_sb = fin.tile([128, 256], f32, tag="o_sb", name="o_sb")
                    for j in range(4):
                        nc.vector.tensor_scalar_mul(
                            o_sb[:, j * 64:(j + 1) * 64],
                            ot[:, j * 65:j * 65 + 64],
                            recip[:, j:j + 1])
                    nc.sync.dma_start(
                        Od[hA + hh, b * 512:(b + 1) * 512, :]
                        .rearrange("(s p) d -> p s d", p=P),
                        o_sb[:].rearrange("p (s d) -> p s d", s=4))

    nc.compile()
    return nc


def _get_nc():
    if "nc" not in _cache:
        _cache["nc"] = _build()
    return _cache["nc"]


def kernel(Q, K, V):
    from concourse.bass_utils import run_bass_kernel_spmd

    Q = np.ascontiguousarray(np.asarray(Q, dtype=np.float32)).reshape(B * H, S, D)
    K = np.ascontiguousarray(np.asarray(K, dtype=np.float32)).reshape(B * H, S, D)
    V = np.ascontiguousarray(np.asarray(V, dtype=np.float32)).reshape(B * H, S, D)

    nc = _get_nc()
    in_maps = [
        {"Q": Q[c * HPC:(c + 1) * HPC],
         "K": K[c * HPC:(c + 1) * HPC],
         "V": V[c * HPC:(c + 1) * HPC]}
        for c in range(NCORES)
    ]
    res = run_bass_kernel_spmd(nc, in_maps, core_ids=list(range(NCORES)))
    out = np.concatenate([res.results[c]["O"] for c in range(NCORES)], axis=0)
    return out.reshape(B, H, S, D)

